# revision 86
# baseline (speedup 1.0000x reference)
"""Trainium2 Bass kernel for nn_Encoder (FSPool set encoder).

Computation per event b (8192 events, data-parallel over 8 cores):
  h = relu(x[b].reshape(128,4) @ W1 + b1)        # per-particle MLP
  h = relu(h @ W2 + b2)
  z = h @ W3 (+ b3)                              # [128 particles, 32 ch]
  z_sorted = sort_desc(z.T, axis=-1)             # per-channel sort over particles
  pooled[c] = sum_p z_sorted[c,p] * w[c,p]       # rank-weighted pool
  mus = pooled[::2]; logvars = pooled[1::2]
  samples = mus + eps * exp(0.5*logvars)

Device layout (per core, 1024 events):
  - MLP runs on TensorE with hidden dim on partitions, particles streaming.
  - z is materialized channel-major: partition = 32*(e%4)+c, so each
    [128, 128] tile holds 4 events x 32 channels of per-particle values.
  - The per-row descending sort is a Batcher odd-even merge network (28
    substages, 1471 comparators vs bitonic's 1792) on VectorE in fp16 with
    the slot axis innermost so every operand stays contiguous for the 2x
    DVE mode; untouched positions cross the ping-pong buffers via the
    parity-minimal _COPY_PLAN (42% of naive volume) in 4x copy mode.
  - b3 never enters the sort: sorting z+const shifts all ranks equally, so
    its pooled contribution b3[c]*sum_p w[c,p] is added as a per-partition
    offset afterwards.

Engine budget (cost model, per core): the DVE sort dominates (~427 us of
compare-exchange at 2 elem/cycle + ~30 us of parity-minimal copy-through,
~469 us busy = 91% of the span); ACT carries the relu PSUM drains (both
layers widened to 1024 to halve per-instruction overhead), the z copies
and the rank-reduces (~418 us); PE ~163 us; GPSIMD the rank-weight
multiply (~65 us). Alternatives
measured WORSE in the timeline model: GPSIMD sort offload (Pool ISA has no
tensor-tensor max — only add/sub/mult — and caps APs at 2 free dims; a
4-instruction emulation runs ~7x slower per element and serializes both
engines), 192/256-event super-tiles (instruction amortization loses to
coarser pipeline boundaries), 1024-wide relus (ACT is not critical), and
moving the rank-reduce to the DVE (it extends the critical engine).
"""

import os
import numpy as np

NCORES = 8
B = 8192
P = 128          # particles per event (set size)
F = 4            # input features per particle
H = 128          # hidden width
C = 32           # 2*LATENT pooled channels
LAT = 16
NPIECES = 20

E = B // NCORES          # events per core
ST_E = 128               # max events per super-tile
NG = ST_E // 4           # max groups of 4 events per super-tile (32)
GALL = E // 4            # total groups per core (stage columns)


def _chunks(e_total):
    """Event counts per super-tile: small head tiles shorten the pipeline
    ramp-in (DVE idle before the first z tile is ready); small tail tiles
    shorten the drain (the last sorts overlap with nothing downstream)."""
    out = [32, 64, 96]
    left = e_total - sum(out) - 64
    while left > 0:
        c = min(ST_E, left)
        out.append(c)
        left -= c
    out.append(64)
    return out


_BUILT = None
LAST_RESULTS = None      # test harness can inspect exec_time_ns / profile


def _oems_copy_plan(p):
    """Minimal ping-pong copy plan for one OEMS stage over a 2p block.

    A position untouched for two consecutive substages lands back in the
    correct ping-pong buffer by parity, so only positions read at the NEXT
    substage from the wrong buffer need copying. Greedy residency
    simulation; returns {substage_index_in_stage: [(start, len), ...]}
    block-relative. Asserts every compare-exchange read and the stage-end
    handoff see the right buffer."""
    n2 = 2 * p
    ks = [p]
    k = p // 2
    while k >= 1:
        ks.append(k)
        k //= 2
    res = [1] * n2          # after the full-touch substage, all in buffer 1
    plan = {}
    for i in range(1, len(ks)):
        k = ks[i]
        rb, wb = i % 2, (i + 1) % 2
        T = set(range(k, n2 - k))
        if i + 1 < len(ks):
            k2 = ks[i + 1]
            nextT = set(range(k2, n2 - k2))
            need = (i + 1) % 2
        else:
            nextT = set(range(n2))
            need = wb
        for x in T:
            assert res[x] == rb
        cps = []
        for x in range(n2):
            if x in T:
                res[x] = wb
            elif (x in nextT or i + 1 == len(ks)) and res[x] != need:
                assert res[x] == rb
                cps.append(x)
                res[x] = wb
        runs = []
        for x in cps:
            if runs and x == runs[-1][0] + runs[-1][1]:
                runs[-1] = (runs[-1][0], runs[-1][1] + 1)
            else:
                runs.append((x, 1))
        if runs:
            plan[i] = runs
    assert all(r == len(ks) % 2 for r in res)
    return plan


_COPY_PLAN = {p: _oems_copy_plan(p) for p in (2, 4, 8, 16, 32, 64)}


def _fspool_interp_matrix():
    """M [21, 128] with w_table = pool_weight @ M (matches reference math)."""
    pos = (np.arange(P, dtype=np.float32) / np.float32(P - 1)) * np.float32(NPIECES)
    idx = np.clip(pos.astype(np.int32), 0, NPIECES)
    frac = pos - idx.astype(np.float32)
    M = np.zeros((NPIECES + 1, P), dtype=np.float32)
    for p in range(P):
        i = int(idx[p])
        M[i, p] += np.float32(1.0) - frac[p]
        M[min(i + 1, NPIECES), p] += frac[p]
    return M


def _build():
    global _BUILT
    if _BUILT is not None:
        return _BUILT
    from contextlib import ExitStack
    import concourse.bass as bass
    import concourse.bacc as bacc
    import concourse.tile as tile
    import concourse.mybir as mybir

    f32 = mybir.dt.float32
    f16 = mybir.dt.float16
    AF = mybir.ActivationFunctionType
    OP = mybir.AluOpType

    nc = bacc.Bacc("TRN2", target_bir_lowering=False, debug=False)

    xt_d = nc.dram_tensor("xt", [F, E * P], f16, kind="ExternalInput")
    w1_d = nc.dram_tensor("w1", [F, H], f16, kind="ExternalInput")
    w2_d = nc.dram_tensor("w2", [H, H], f16, kind="ExternalInput")
    w3_d = nc.dram_tensor("w3", [H, C], f16, kind="ExternalInput")
    b1_d = nc.dram_tensor("b1", [H, 1], f32, kind="ExternalInput")
    b2_d = nc.dram_tensor("b2", [H, 1], f32, kind="ExternalInput")
    wrept_d = nc.dram_tensor("wrept", [128, NG, P], f16, kind="ExternalInput")
    offs_d = nc.dram_tensor("offs", [128, 1], f32, kind="ExternalInput")
    epst_d = nc.dram_tensor("epst", [64, GALL], f32, kind="ExternalInput")

    mus_d = nc.dram_tensor("mus_t", [64, GALL], f32, kind="ExternalOutput")
    lv_d = nc.dram_tensor("logvars_t", [64, GALL], f32, kind="ExternalOutput")
    smp_d = nc.dram_tensor("samples_t", [64, GALL], f32, kind="ExternalOutput")

    with tile.TileContext(nc) as tc:
        with ExitStack() as ctx:
            consts = ctx.enter_context(tc.tile_pool(name="consts", bufs=1))
            xpool = ctx.enter_context(tc.tile_pool(name="x", bufs=2))
            hpool = ctx.enter_context(tc.tile_pool(name="h", bufs=4))
            zpool = ctx.enter_context(tc.tile_pool(name="z", bufs=3))
            zbpool = ctx.enter_context(tc.tile_pool(name="zb", bufs=2))
            ppool = ctx.enter_context(tc.tile_pool(name="prodp", bufs=3))
            spool = ctx.enter_context(tc.tile_pool(name="stage", bufs=1))
            epool = ctx.enter_context(tc.tile_pool(name="epi", bufs=1))
            ps1 = ctx.enter_context(tc.tile_pool(name="ps1", bufs=2, space="PSUM"))
            ps2 = ctx.enter_context(tc.tile_pool(name="ps2", bufs=1, space="PSUM"))
            ps3 = ctx.enter_context(tc.tile_pool(name="ps3", bufs=2, space="PSUM"))

            w1_s = consts.tile([F, H], f16)
            nc.sync.dma_start(out=w1_s[:], in_=w1_d[:])
            w2_s = consts.tile([H, H], f16)
            nc.sync.dma_start(out=w2_s[:], in_=w2_d[:])
            w3_s = consts.tile([H, C], f16)
            nc.sync.dma_start(out=w3_s[:], in_=w3_d[:])
            b1_s = consts.tile([H, 1], f32)
            nc.sync.dma_start(out=b1_s[:], in_=b1_d[:])
            b2_s = consts.tile([H, 1], f32)
            nc.sync.dma_start(out=b2_s[:], in_=b2_d[:])
            wrept_s = consts.tile([128, NG, P], f16)
            nc.sync.dma_start(out=wrept_s[:], in_=wrept_d[:])
            offs_s = consts.tile([128, 1], f32)
            nc.sync.dma_start(out=offs_s[:], in_=offs_d[:])

            stage = spool.tile([128, GALL], f32)
            rscr = spool.tile([128, P], f16, tag="rscr")

            ev0 = 0
            pending = None
            pend_mult = None
            chunks = _chunks(E)
            for st_i, st_e in enumerate(chunks):
                ng = st_e // 4
                col0 = ev0 // 4
                last = st_i == len(chunks) - 1
                # during the pipeline ramp the DVE is idle (nothing to sort
                # yet) — route half the first chunk's relus there so the
                # first z tile is ready sooner
                ramp = st_i == 0
                # stage x in 96-event sub-chunks: a whole 192-event tile on
                # the 4 feature partitions would not leave room in SBUF
                xt_tiles = []
                for x0 in range(0, st_e, 128):
                    xe = min(128, st_e - x0)
                    xt_s = xpool.tile([F, xe * P], f16, tag="xt")
                    nc.sync.dma_start(
                        out=xt_s[:],
                        in_=xt_d[:, (ev0 + x0) * P:(ev0 + x0 + xe) * P],
                    )
                    xt_tiles.append(xt_s)
                zA = zpool.tile([128, P, ng], f16, tag="zA")
                zB = zbpool.tile([128, P, ng], f16, tag="zB")

                for gq in range(ng // 4):
                    # one PSUM bank collects z for 4 groups of 4 events
                    pz = ps3.tile([128, 4 * P], f32, tag="pz")
                    for pp2 in range(2):
                        # relu1 drains two groups per instruction (the ACT
                        # per-instruction overhead was gating the pipeline
                        # cadence); ph1 spans two PSUM banks, halves written
                        # by separate matmuls
                        g0 = gq * 4 + 2 * pp2
                        ph1 = ps1.tile([128, 1024], f32, tag="ph1")
                        for gg2 in range(2):
                            g = g0 + gg2
                            xt_s = xt_tiles[g // 32]
                            gl = g % 32
                            nc.tensor.matmul(
                                ph1[:, gg2 * 512:(gg2 + 1) * 512], w1_s[:],
                                xt_s[:, gl * 512:(gl + 1) * 512],
                                start=True, stop=True,
                            )
                        h1w = hpool.tile([128, 1024], f16, tag="h1")
                        if ramp and pp2 == 0:
                            nc.vector.tensor_scalar(
                                out=h1w[:], in0=ph1[:], scalar1=b1_s[:],
                                scalar2=0.0, op0=OP.add, op1=OP.max)
                        else:
                            nc.scalar.activation(h1w[:], ph1[:], AF.Relu,
                                                 bias=b1_s[:])
                        ph2 = ps2.tile([128, 1024], f32, tag="ph2")
                        for gg2 in range(2):
                            nc.tensor.matmul(
                                ph2[:, gg2 * 512:(gg2 + 1) * 512], w2_s[:],
                                h1w[:, gg2 * 512:(gg2 + 1) * 512],
                                start=True, stop=True)
                        h2w = hpool.tile([128, 1024], f16, tag="h2")
                        if ramp and pp2 == 1:
                            nc.vector.tensor_scalar(
                                out=h2w[:], in0=ph2[:], scalar1=b2_s[:],
                                scalar2=0.0, op0=OP.add, op1=OP.max)
                        else:
                            nc.scalar.activation(h2w[:], ph2[:], AF.Relu,
                                                 bias=b2_s[:])

                        for gg2 in range(2):
                            gg = 2 * pp2 + gg2
                            for e4 in range(4):
                                nc.tensor.matmul(
                                    pz[32 * e4:32 * (e4 + 1),
                                       gg * P:(gg + 1) * P],
                                    w3_s[:],
                                    h2w[:, gg2 * 512 + e4 * P:
                                        gg2 * 512 + (e4 + 1) * P],
                                    start=True, stop=True,
                                    tile_position=(0, 32 * e4),
                                )
                    # batched copy+cast into slot-interleaved sort tile
                    nc.scalar.activation(
                        zA[:, :, 4 * gq:4 * (gq + 1)].rearrange("a p s -> a s p"),
                        pz[:].rearrange("a (s p) -> a s p", p=P),
                        AF.Copy,
                    )

                # ---- Batcher odd-even merge sort (descending), ping-pong ----
                # 18% fewer compare-exchange elements than bitonic and no
                # reversed accesses. Untouched positions mostly need NO
                # copy-through: two consecutive skips return a position to
                # the correct ping-pong buffer by parity, so only the
                # precomputed _COPY_PLAN bands (42% of the naive volume)
                # are moved, in 4x copy mode.
                cur, other = zA, zB
                for pp in (1, 2, 4, 8, 16, 32, 64):
                    k = pp
                    si = 0
                    while k >= 1:
                        kn = k * ng
                        m2 = 2 * pp * ng
                        if k == pp:
                            va = cur[:].rearrange("p a s -> p (a s)").rearrange(
                                "p (nb m) -> p nb m", m=2 * kn)
                            vb = other[:].rearrange("p a s -> p (a s)").rearrange(
                                "p (nb m) -> p nb m", m=2 * kn)
                            nc.vector.tensor_tensor(
                                out=vb[:, :, 0:kn], in0=va[:, :, 0:kn],
                                in1=va[:, :, kn:2 * kn], op=OP.max)
                            nc.vector.tensor_tensor(
                                out=vb[:, :, kn:2 * kn], in0=va[:, :, kn:2 * kn],
                                in1=va[:, :, 0:kn], op=OP.min)
                        else:
                            nq = pp // k
                            va = cur[:].rearrange("p a s -> p (a s)").rearrange(
                                "p (nb q m) -> p nb q m", q=nq, m=2 * kn)
                            vb = other[:].rearrange("p a s -> p (a s)").rearrange(
                                "p (nb q m) -> p nb q m", q=nq, m=2 * kn)
                            nc.vector.tensor_tensor(
                                out=vb[:, :, 0:nq - 1, kn:2 * kn],
                                in0=va[:, :, 0:nq - 1, kn:2 * kn],
                                in1=va[:, :, 1:nq, 0:kn], op=OP.max)
                            nc.vector.tensor_tensor(
                                out=vb[:, :, 1:nq, 0:kn],
                                in0=va[:, :, 1:nq, 0:kn],
                                in1=va[:, :, 0:nq - 1, kn:2 * kn], op=OP.min)
                            for cst, cln in _COPY_PLAN[pp].get(si, []):
                                vaf = cur[:].rearrange(
                                    "p a s -> p (a s)").rearrange(
                                    "p (nb m) -> p nb m", m=m2)
                                vbf = other[:].rearrange(
                                    "p a s -> p (a s)").rearrange(
                                    "p (nb m) -> p nb m", m=m2)
                                nc.vector.tensor_copy(
                                    out=vbf[:, :, cst * ng:(cst + cln) * ng],
                                    in_=vaf[:, :, cst * ng:(cst + cln) * ng])
                        cur, other = other, cur
                        si += 1
                        k //= 2

                # ---- rank-weighted pooling: prod[c, g, p] = z_sorted * w
                # (GPSIMD; DVE fp16-2x for the tail chunk where GPSIMD
                # latency would sit on the critical drain path), then reduce
                # over p into stage columns (ACT Copy+accum, deferred one
                # chunk so it never waits on the multiply).
                prod = ppool.tile([128, ng, P], f16, tag="prod")
                if last:
                    nc.vector.tensor_tensor(
                        out=prod[:],
                        in0=cur[:].rearrange("a p s -> a s p"),
                        in1=wrept_s[:, 0:ng, :],
                        op=OP.mult,
                    )
                else:
                    q = max(ng // 4, 1)
                    for qi in range(0, ng, q):
                        qe = min(q, ng - qi)
                        nc.gpsimd.tensor_tensor(
                            out=prod[:, qi:qi + qe, :],
                            in0=cur[:, :, qi:qi + qe].rearrange(
                                "a p s -> a s p"),
                            in1=wrept_s[:, qi:qi + qe, :],
                            op=OP.mult,
                        )
                if pending is not None:
                    p_prod, p_col, p_ng = pending
                    for s in range(0, p_ng):
                        nc.scalar.activation(
                            rscr[:], p_prod[:, s, :], AF.Copy,
                            accum_out=stage[:, p_col + s:p_col + s + 1],
                        )
                pending = (prod, col0, ng)
                ev0 += st_e

            # ---- epilogue, split by column range ----
            # channels are host-permuted so that within each 32-partition
            # block, partitions 0:16 hold mus and 16:32 hold logvars. The
            # columns of every chunk but the last are final once the last
            # rscr burst ran, so their epilogue (shuffle DMAs, exp, sampling,
            # output DMAs) overlaps the last chunk's sort; only the last
            # chunk's columns pay the serial drain after the final reduce.
            mus_al = epool.tile([64, GALL], f32)
            lv_al = epool.tile([64, GALL], f32)
            ex = epool.tile([64, GALL], f32)
            eps_s = epool.tile([64, GALL], f32)
            smp = epool.tile([64, GALL], f32)
            nc.sync.dma_start(out=eps_s[:], in_=epst_d[:])

            def emit_epilogue(c0, c1):
                nc.vector.tensor_scalar_add(
                    stage[:, c0:c1], stage[:, c0:c1], offs_s[:])
                for q in range(4):
                    nc.sync.dma_start(
                        out=mus_al[16 * q:16 * (q + 1), c0:c1],
                        in_=stage[32 * q:32 * q + 16, c0:c1],
                    )
                    nc.sync.dma_start(
                        out=lv_al[16 * q:16 * (q + 1), c0:c1],
                        in_=stage[32 * q + 16:32 * q + 32, c0:c1],
                    )
                nc.scalar.activation(ex[:, c0:c1], lv_al[:, c0:c1],
                                     AF.Exp, scale=0.5)
                nc.vector.tensor_tensor(
                    out=smp[:, c0:c1], in0=eps_s[:, c0:c1],
                    in1=ex[:, c0:c1], op=OP.mult)
                nc.vector.tensor_tensor(
                    out=smp[:, c0:c1], in0=smp[:, c0:c1],
                    in1=mus_al[:, c0:c1], op=OP.add)
                nc.sync.dma_start(out=mus_d[:, c0:c1], in_=mus_al[:, c0:c1])
                nc.sync.dma_start(out=lv_d[:, c0:c1], in_=lv_al[:, c0:c1])
                nc.sync.dma_start(out=smp_d[:, c0:c1], in_=smp[:, c0:c1])

            p_prod, p_col, p_ng = pending
            emit_epilogue(0, p_col)

            # final chunk: reduce on the DVE (idle at the drain) instead of
            # queueing behind ACT, then drain its columns
            nc.vector.tensor_reduce(
                out=stage[:, p_col:p_col + p_ng],
                in_=p_prod[:],
                axis=mybir.AxisListType.X,
                op=OP.add,
            )
            emit_epilogue(p_col, GALL)

    nc.compile()
    _BUILT = nc
    return nc


def _host_prep(x, W1, b1, W2, b2, W3, b3, pool_weight, eps):
    x = np.asarray(x, np.float32)
    eps = np.asarray(eps, np.float32)
    W1 = np.asarray(W1, np.float32).astype(np.float16)
    W2 = np.asarray(W2, np.float32).astype(np.float16)
    W3 = np.asarray(W3, np.float32)
    b1 = np.asarray(b1, np.float32).reshape(H, 1)
    b2 = np.asarray(b2, np.float32).reshape(H, 1)
    b3 = np.asarray(b3, np.float32)
    pw = np.asarray(pool_weight, np.float32)

    # channel permutation: device channel c' maps to logical channel perm[c']
    # (mus channels 0,2,..,30 first, then logvar channels 1,3,..,31)
    perm = np.concatenate([np.arange(0, C, 2), np.arange(1, C, 2)])
    W3 = np.ascontiguousarray(W3[:, perm]).astype(np.float16)
    b3p = b3[perm]
    w_table = (pw @ _fspool_interp_matrix()).astype(np.float32)[perm]  # [32, 128]
    wrep = np.tile(w_table, (4, 1))                                    # [128, 128]
    wrept = np.ascontiguousarray(
        np.broadcast_to(wrep[:, None, :], (128, NG, P))
    ).astype(np.float16)
    offs = np.tile(b3p * w_table.sum(axis=1), 4).reshape(128, 1).astype(np.float32)

    in_maps = []
    for c in range(NCORES):
        xs = x[c * E:(c + 1) * E]                                  # [E, 512]
        xt = np.ascontiguousarray(
            xs.reshape(E, P, F).transpose(2, 0, 1).reshape(F, E * P)
        ).astype(np.float16)
        es = eps[c * E:(c + 1) * E]                                # [E, 16]
        epst = np.ascontiguousarray(
            es.reshape(GALL, 4, LAT).transpose(1, 2, 0).reshape(64, GALL)
        )
        in_maps.append({
            "xt": xt, "w1": W1, "w2": W2, "w3": W3,
            "b1": b1, "b2": b2, "wrept": wrept, "offs": offs, "epst": epst,
        })
    return in_maps


def _host_post(results):
    mus = np.empty((B, LAT), np.float32)
    logvars = np.empty((B, LAT), np.float32)
    samples = np.empty((B, LAT), np.float32)
    for c, r in enumerate(results):
        for name, dst in (("mus_t", mus), ("logvars_t", logvars),
                          ("samples_t", samples)):
            t = r[name].reshape(4, LAT, GALL).transpose(2, 0, 1).reshape(E, LAT)
            dst[c * E:(c + 1) * E] = t
    return mus, logvars, samples


def kernel(**inputs):
    global LAST_RESULTS
    from concourse.bass_utils import run_bass_kernel_spmd

    nc = _build()
    in_maps = _host_prep(**inputs)
    trace = bool(int(os.environ.get("KERNEL_TRACE", "0")))
    res = run_bass_kernel_spmd(nc, in_maps, list(range(NCORES)), trace=trace)
    LAST_RESULTS = res
    return _host_post(res.results)
